# revision 1
# baseline (speedup 1.0000x reference)
"""MetapathAttentionLayer Trainium2 kernel.

Math (per node n):
    scores[n, m] = sum_d x[m, n, d] * W[d, m]
    att = softmax(relu(scores), axis=m)      (8 metapaths)
    out[n, :] = elu(sum_m att[n, m] * x[m, n, :])

Strategy: shard nodes across 8 cores (data parallel). Per core, natural
layout [nodes(part), d(free)] in bf16:
  - scores: DVE tensor_tensor mul vs replicated-W tile + tensor_scalar
    accum_out reductions (fused sum over d)
  - softmax: exp(relu(s)) == max(exp(s), 1); ACT Exp + DVE max/sum/recip
  - pooling: PE matmuls with diag(att_m) stationary (built by GPSIMD
    local_scatter / ACT tensor_tensor on identity blocks), accumulating
    over m into PSUM
  - elu(x) = relu(x) + exp(min(x, 0)) - 1 composed on ACT
"""

import os
from contextlib import ExitStack

import numpy as np
import ml_dtypes

import concourse.bass as bass
import concourse.tile as tile
from concourse import bacc, mybir, library_config
import concourse.bass_utils as bass_utils

F32 = mybir.dt.float32
BF16 = mybir.dt.bfloat16
I16 = mybir.dt.int16
ALU = mybir.AluOpType
ACTF = mybir.ActivationFunctionType

NMETA = 8
N = 100000
D = 128
NCORES = 8
NC_RAW = N // NCORES          # 12500 nodes per core
CHUNK = 128                   # nodes per compute chunk (partition dim)
NC_PAD = 12544                # 98 chunks of 128
T_CHUNKS = 8                  # chunks per DMA T-tile (1024 nodes)
GROUP = 4                     # chunks per PSUM/elu group (psum bank = 512 f32)

# tunables
DIAG_DVE_EVERY = 3   # every k-th chunk builds diag via DVE tensor_scalar (0=off)


def kernel_body(tc, out_d, x_d, wb_d, sidx_d, icat_d,
                nc_pad=NC_PAD, t_chunks=T_CHUNKS, reps=1,
                diag_dve_every=DIAG_DVE_EVERY, comb_on_pool=False):
    nc = tc.nc
    with ExitStack() as ctx:
        const = ctx.enter_context(tc.tile_pool(name="const", bufs=1))
        xpool = ctx.enter_context(tc.tile_pool(name="x", bufs=3))
        opool = ctx.enter_context(tc.tile_pool(name="o", bufs=2))
        ppool = ctx.enter_context(tc.tile_pool(name="prod", bufs=3))
        tpool = ctx.enter_context(tc.tile_pool(name="trash", bufs=2))
        spool = ctx.enter_context(tc.tile_pool(name="smalls", bufs=6))
        dpool = ctx.enter_context(tc.tile_pool(name="diag", bufs=6))
        epool = ctx.enter_context(tc.tile_pool(name="elu", bufs=3))
        psum = ctx.enter_context(tc.tile_pool(name="ps", bufs=6, space="PSUM"))

        wb = const.tile([128, NMETA * D], BF16)
        nc.sync.dma_start(wb[:], wb_d[:])
        sidx = const.tile([128, NMETA], I16)
        nc.sync.dma_start(sidx[:], sidx_d[:])
        icat = const.tile([128, NMETA * D], BF16)
        nc.sync.dma_start(icat[:], icat_d[:])
        nc.gpsimd.load_library(library_config.local_scatter)

        chunk_idx = 0
        for _rep in range(reps):
            n0 = 0
            while n0 < nc_pad:
                ct = min(t_chunks, (nc_pad - n0) // CHUNK)
                nt = ct * CHUNK

                # node n = n0 + p*ct + c  ->  partition p, free chunk c
                X = xpool.tile([128, NMETA * nt], BF16, tag="X")
                for m in range(NMETA):
                    src = x_d[m, n0:n0 + nt, :].rearrange(
                        "(p c) d -> p (c d)", p=128)
                    nc.sync.dma_start(X[:, m * nt:(m + 1) * nt], src)
                Xv = X[:].rearrange("p (m c d) -> p m c d", m=NMETA, c=ct)

                out_sb = opool.tile([128, nt], F32, tag="osb")

                for g0 in range(0, ct, GROUP):
                    gl = min(GROUP, ct - g0)
                    ps = psum.tile([128, GROUP * D], F32, tag="ps")
                    scores = spool.tile([128, GROUP * NMETA], F32, tag="scores")

                    # one batched multiply for the whole group of chunks
                    P = ppool.tile([128, NMETA * GROUP * D], BF16, tag="P")
                    Pv = P[:].rearrange("p (m c d) -> p m c d", m=NMETA, c=GROUP)
                    nc.vector.tensor_tensor(
                        out=Pv[:, :, :gl, :],
                        in0=Xv[:, :, g0:g0 + gl, :],
                        in1=wb[:].rearrange("p (m d) -> p m d", m=NMETA)
                              .unsqueeze(2).broadcast_to([128, NMETA, gl, D]),
                        op=ALU.mult,
                    )
                    tr = tpool.tile([128, D], BF16, tag="tr")
                    for cg in range(gl):
                        for m in range(NMETA):
                            nc.vector.tensor_scalar(
                                tr[:],
                                Pv[:, m, cg, :],
                                1.0,
                                None,
                                ALU.mult,
                                ALU.add,
                                accum_out=scores[:, cg * NMETA + m:
                                                 cg * NMETA + m + 1],
                            )

                    # softmax over m: att = e/sum(e), e = exp(relu(s)) = max(exp(s),1)
                    e_raw = spool.tile([128, GROUP * NMETA], F32, tag="eraw")
                    nc.scalar.activation(
                        e_raw[:, :gl * NMETA], scores[:, :gl * NMETA], ACTF.Exp)
                    e_bf = spool.tile([128, GROUP * NMETA], BF16, tag="ebf")
                    nc.vector.tensor_scalar(
                        e_bf[:, :gl * NMETA], e_raw[:, :gl * NMETA],
                        1.0, None, ALU.max)
                    sums = spool.tile([128, GROUP], F32, tag="sums")
                    nc.vector.tensor_reduce(
                        out=sums[:, :gl],
                        in_=e_bf[:, :gl * NMETA].rearrange(
                            "p (c m) -> p c m", m=NMETA),
                        axis=mybir.AxisListType.X,
                        op=ALU.add,
                    )
                    inv = spool.tile([128, GROUP], F32, tag="inv")
                    nc.vector.reciprocal(inv[:, :gl], sums[:, :gl])

                    for cg in range(gl):
                        c = g0 + cg
                        diag = dpool.tile([128, NMETA * D], BF16, tag="diag")
                        use_dve = (diag_dve_every and
                                   chunk_idx % diag_dve_every == 0)
                        if use_dve:
                            att_f = spool.tile([128, NMETA], F32, tag="attf")
                            nc.vector.tensor_scalar(
                                att_f[:], e_bf[:, cg * NMETA:(cg + 1) * NMETA],
                                inv[:, cg:cg + 1], None, ALU.mult)
                            for m in range(NMETA):
                                nc.vector.tensor_scalar(
                                    diag[:, m * D:(m + 1) * D],
                                    icat[:, m * D:(m + 1) * D],
                                    att_f[:, m:m + 1], None, ALU.mult)
                        else:
                            att = spool.tile([128, NMETA], BF16, tag="att")
                            nc.vector.tensor_scalar(
                                att[:], e_bf[:, cg * NMETA:(cg + 1) * NMETA],
                                inv[:, cg:cg + 1], None, ALU.mult)
                            nc.gpsimd.local_scatter(
                                diag[:], att[:], sidx[:],
                                channels=128, num_elems=NMETA * D,
                                num_idxs=NMETA)
                        for m in range(NMETA):
                            nc.tensor.matmul(
                                out=ps[:, cg * D:(cg + 1) * D],
                                lhsT=diag[:, m * D:(m + 1) * D],
                                rhs=Xv[:, m, c, :],
                                start=(m == 0),
                                stop=(m == NMETA - 1),
                            )
                        chunk_idx += 1

                    # elu(x) = relu(x) + exp(min(x,0)) - 1
                    w = gl * D
                    r = epool.tile([128, GROUP * D], F32, tag="r")
                    nc.scalar.activation(r[:, :w], ps[:, :w], ACTF.Relu)
                    t = epool.tile([128, GROUP * D], F32, tag="t")
                    nc.scalar.activation(t[:, :w], ps[:, :w], ACTF.Relu,
                                         scale=-1.0)
                    e2 = epool.tile([128, GROUP * D], F32, tag="e2")
                    nc.scalar.activation(e2[:, :w], t[:, :w], ACTF.Exp,
                                         scale=-1.0)
                    # out = (e2 + (-1)) + r  in one fused op
                    eng = nc.gpsimd if comb_on_pool else nc.vector
                    eng.scalar_tensor_tensor(
                        out=out_sb[:, g0 * D:g0 * D + w],
                        in0=e2[:, :w], scalar=-1.0, in1=r[:, :w],
                        op0=ALU.add, op1=ALU.add)

                dsto = out_d[n0:n0 + nt, :].rearrange("(p c) d -> p (c d)", p=128)
                nc.sync.dma_start(dsto, out_sb[:])
                n0 += nt


def host_inputs(x_np, w_np, nc_pad=NC_PAD):
    """Build per-core input maps from full fp32 inputs."""
    in_maps = []
    wbig = np.ascontiguousarray(
        np.broadcast_to(w_np.T.reshape(1, NMETA * D), (128, NMETA * D))
    ).astype(ml_dtypes.bfloat16)
    sidx = (np.arange(NMETA)[None, :] * D + np.arange(128)[:, None]).astype(np.int16)
    icat = np.ascontiguousarray(
        np.tile(np.eye(128, dtype=np.float32), (1, NMETA))
    ).astype(ml_dtypes.bfloat16)
    nc_raw = x_np.shape[1] // NCORES
    for c in range(NCORES):
        xs = x_np[:, c * nc_raw:(c + 1) * nc_raw, :]
        xp = np.zeros((NMETA, nc_pad, D), dtype=ml_dtypes.bfloat16)
        xp[:, :nc_raw, :] = xs.astype(ml_dtypes.bfloat16)
        in_maps.append({"x": xp, "wb": wbig, "sidx": sidx, "icat": icat})
    return in_maps


_CACHE = {}


def build(reps=1, **kw):
    key = (reps, tuple(sorted(kw.items())))
    if key in _CACHE:
        return _CACHE[key]
    nc = bacc.Bacc("TRN2", target_bir_lowering=False, debug=False,
                   num_devices=NCORES)
    x = nc.dram_tensor("x", [NMETA, NC_PAD, D], BF16, kind="ExternalInput").ap()
    wb = nc.dram_tensor("wb", [128, NMETA * D], BF16, kind="ExternalInput").ap()
    sidx = nc.dram_tensor("sidx", [128, NMETA], I16, kind="ExternalInput").ap()
    icat = nc.dram_tensor("icat", [128, NMETA * D], BF16, kind="ExternalInput").ap()
    out = nc.dram_tensor("out", [NC_PAD, D], F32, kind="ExternalOutput").ap()
    with tile.TileContext(nc) as tc:
        kernel_body(tc, out, x, wb, sidx, icat, reps=reps, **kw)
    nc.compile()
    _CACHE[key] = nc
    return nc


def run(input, W, trace=False, **trace_kwargs):
    x_np = np.asarray(input, dtype=np.float32)
    w_np = np.asarray(W, dtype=np.float32)
    nc = build()
    in_maps = host_inputs(x_np, w_np)
    res = bass_utils.run_bass_kernel_spmd(
        nc, in_maps, core_ids=list(range(NCORES)), trace=trace, **trace_kwargs)
    nc_raw = x_np.shape[1] // NCORES
    full = np.concatenate(
        [res.results[c]["out"][:nc_raw] for c in range(NCORES)], axis=0)
    return full, res


def kernel(input, W):
    out, _ = run(input, W, trace=False)
    return out


# ---------------------------------------------------------------------------
# Timing harness (test-only): persistent jit over the bass_exec primitive so
# repeated executions reuse device-resident inputs. HW kernel time is derived
# from the slope between an R-repeat NEFF and the 1-repeat NEFF.
# ---------------------------------------------------------------------------

def make_runner(nc):
    import jax
    from jax.experimental.shard_map import shard_map
    from jax.sharding import Mesh, PartitionSpec, NamedSharding
    from concourse import bass2jax as b2j

    b2j.install_neuronx_cc_hook()
    partition_name = nc.partition_id_tensor.name if nc.partition_id_tensor else None
    in_names, out_names, out_avals, zero_outs = [], [], [], []
    for alloc in nc.m.functions[0].allocations:
        if not isinstance(alloc, mybir.MemoryLocationSet):
            continue
        name = alloc.memorylocations[0].name
        if alloc.kind == "ExternalInput":
            if name != partition_name:
                in_names.append(name)
        elif alloc.kind == "ExternalOutput":
            out_names.append(name)
            shape = tuple(alloc.tensor_shape)
            dtype = mybir.dt.np(alloc.dtype)
            out_avals.append(jax.core.ShapedArray(shape, dtype))
            zero_outs.append(np.zeros(shape, dtype))
    n_params = len(in_names)
    n_outs = len(out_avals)
    all_names = in_names + out_names + ([partition_name] if partition_name else [])

    def _body(*args):
        operands = list(args)
        if partition_name is not None:
            operands.append(b2j.partition_id_tensor())
        outs = b2j._bass_exec_p.bind(
            *operands,
            out_avals=tuple(out_avals),
            in_names=tuple(all_names),
            out_names=tuple(out_names),
            lowering_input_output_aliases=(),
            sim_require_finite=True,
            sim_require_nnan=True,
            nc=nc,
        )
        return tuple(outs)

    devices = jax.devices()[:NCORES]
    mesh = Mesh(np.asarray(devices), ("core",))
    in_specs = (PartitionSpec("core"),) * (n_params + n_outs)
    out_specs = (PartitionSpec("core"),) * n_outs
    donate = tuple(range(n_params, n_params + n_outs))
    sharded = jax.jit(
        shard_map(_body, mesh=mesh, in_specs=in_specs, out_specs=out_specs,
                  check_rep=False),
        donate_argnums=donate, keep_unused=True)
    sharding = NamedSharding(mesh, PartitionSpec("core"))
    return sharded, in_names, zero_outs, sharding


class _TimedRunner:
    def __init__(self, nc, in_maps):
        import jax
        self.jax = jax
        sharded, in_names, zero_outs, sharding = make_runner(nc)
        self.sharded = sharded
        concat_in = [
            np.concatenate([in_maps[c][n] for c in range(NCORES)], axis=0)
            for n in in_names
        ]
        self.xs = [jax.device_put(a, sharding) for a in concat_in]
        self.zero_outs = zero_outs
        self.sharding = sharding

    def _zset(self):
        return [
            self.jax.device_put(
                np.zeros((NCORES * z.shape[0], *z.shape[1:]), z.dtype),
                self.sharding)
            for z in self.zero_outs
        ]

    def piped(self, reps):
        import time as _t
        zsets = [self._zset() for _ in range(reps + 1)]
        self.jax.block_until_ready(zsets)
        self.jax.block_until_ready(self.xs)
        o = self.sharded(*self.xs, *zsets[0])
        self.jax.block_until_ready(o)
        _ = self.jax.device_get(o[0])
        t0 = _t.perf_counter()
        outs = [self.sharded(*self.xs, *zsets[1 + k]) for k in range(reps)]
        self.jax.block_until_ready(outs)
        # force true device completion: fetch the last output's bytes
        _ = self.jax.device_get(outs[-1][0])
        return (_t.perf_counter() - t0) / reps


def measure(input, W, reps=12, neff_reps=9, rounds=4, **kw):
    """Estimate per-iteration HW time via multi-repeat NEFF slope.

    Interleaves rounds of (1-repeat NEFF, R-repeat NEFF) piped timings and
    takes the min across rounds for each to reject dispatch-overhead noise.
    """
    x_np = np.asarray(input, dtype=np.float32)
    w_np = np.asarray(W, dtype=np.float32)
    in_maps = host_inputs(x_np, w_np)

    nc1 = build(reps=1, **kw)
    ncr = build(reps=neff_reps, **kw)
    r1 = _TimedRunner(nc1, in_maps)
    rr = _TimedRunner(ncr, in_maps)
    t1s, trs = [], []
    for _ in range(rounds):
        t1s.append(r1.piped(reps))
        trs.append(rr.piped(reps))
    t1, tr = min(t1s), min(trs)
    slope = (tr - t1) / (neff_reps - 1)
    return t1, tr, slope, t1s, trs



# revision 6
# speedup vs baseline: 1.0370x; 1.0370x over previous
"""MetapathAttentionLayer Trainium2 kernel (v2: packed node-metapath layout).

Math (per node n):
    scores[n, m] = sum_d x[m, n, d] * W[d, m]
    att = softmax(relu(scores), axis=m)      (8 metapaths)
    out[n, :] = elu(sum_m att[n, m] * x[m, n, :])

Strategy: shard nodes across 8 cores (data parallel).  Per core, nodes are
packed so SBUF partition p = (node%32)*4 + metapath' holds one (node,
metapath) row of x, split into two halves (metapaths 0-3 / 4-7).  Per
region of 1024 nodes (32 tiles of 32 nodes):
  - scores: DVE tensor_tensor multiply against a replicated-W pattern
    (per-partition W column), then a batched binary-tree reduction over d
    (all tree levels are single DVE ops covering every tile).
  - softmax over m: ACT relu+exp; sum over the 8 metapaths of each node
    via PE matmul with a constant block-indicator stationary; DVE
    reciprocal; broadcast back with a second const matmul; weights applied
    on GPSIMD.
  - pooling: GPSIMD local_scatter packs attention weights into 32-wide
    stationaries (4 diagonals each); PE matmuls contract the (node,
    metapath) partition dim, accumulating both halves into PSUM.
  - elu(x) = relu(x) + exp(-relu(-x)) - 1: ACT x3 + GPSIMD combine,
    bf16 output DMA.
"""

from contextlib import ExitStack

import numpy as np
import ml_dtypes

import concourse.bass as bass
import concourse.tile as tile
from concourse import bacc, mybir, library_config
import concourse.bass_utils as bass_utils

F32 = mybir.dt.float32
BF16 = mybir.dt.bfloat16
I16 = mybir.dt.int16
ALU = mybir.AluOpType
ACTF = mybir.ActivationFunctionType

NMETA = 8
N = 100000
D = 128
NCORES = 8
NC_RAW = N // NCORES          # 12500 nodes per core
NC_PAD = 12800                # 400 tiles of 32 nodes
NTILES = NC_PAD // 32         # 400
RTILE = 32                    # tiles per region (1024 nodes)


def _region_sizes():
    """Tiles per region: 12 full regions of 32 tiles + 1 region of 16."""
    sizes = []
    t = NTILES
    while t > 0:
        s = min(RTILE, t)
        sizes.append(s)
        t -= s
    return sizes


def kernel_body(tc, out_d, xa_d, xb_d, wba_d, wbb_d, blk4_d, blk4t_d, sidx_d):
    nc = tc.nc
    with ExitStack() as ctx:
        const = ctx.enter_context(tc.tile_pool(name="const", bufs=1))
        xpool = ctx.enter_context(tc.tile_pool(name="x", bufs=2))
        ppool = ctx.enter_context(tc.tile_pool(name="prod", bufs=2))
        tpool = ctx.enter_context(tc.tile_pool(name="tree", bufs=2))
        spool = ctx.enter_context(tc.tile_pool(name="smalls", bufs=3))
        scat = ctx.enter_context(tc.tile_pool(name="scat", bufs=2))
        epool = ctx.enter_context(tc.tile_pool(name="elu", bufs=2))
        opool = ctx.enter_context(tc.tile_pool(name="osb", bufs=2))
        psum = ctx.enter_context(tc.tile_pool(name="ps", bufs=2, space="PSUM"))
        psum_s = ctx.enter_context(tc.tile_pool(name="pss", bufs=2, space="PSUM"))

        wba = const.tile([128, D], BF16)
        nc.sync.dma_start(wba[:], wba_d[:])
        wbb = const.tile([128, D], BF16)
        nc.sync.dma_start(wbb[:], wbb_d[:])
        blk4 = const.tile([128, 32], BF16)
        nc.sync.dma_start(blk4[:], blk4_d[:])
        blk4t = const.tile([32, 128], F32)
        nc.sync.dma_start(blk4t[:], blk4t_d[:])
        sidx = const.tile([128, RTILE], I16)
        nc.sync.dma_start(sidx[:], sidx_d[:])
        nc.gpsimd.load_library(library_config.local_scatter)

        t0 = 0
        for r, nt in enumerate(_region_sizes()):
            nn = nt * 32          # nodes in region
            fw = nt * D           # free width of an X half

            X = {}
            for h, x_d in (("a", xa_d), ("b", xb_d)):
                X[h] = xpool.tile([128, RTILE * D], BF16, tag=f"X{h}", name=f"X{h}")
                nc.sync.dma_start(
                    X[h][:, :fw],
                    x_d[:, t0:t0 + nt, :].rearrange("p t d -> p (t d)"))

            # ---- scores: multiply then tree-reduce over d --------------
            s = spool.tile([128, 2 * RTILE], F32, tag="s")
            for hi, (h, wb) in enumerate((("a", wba), ("b", wbb))):
                P = ppool.tile([128, RTILE * D], BF16, tag=f"P{h}", name=f"P{h}")
                nc.vector.tensor_tensor(
                    out=P[:, :fw].rearrange("p (t d) -> p t d", t=nt),
                    in0=X[h][:, :fw].rearrange("p (t d) -> p t d", t=nt),
                    in1=wb[:].unsqueeze(1).broadcast_to([128, nt, D]),
                    op=ALU.mult,
                )
                cur = P
                w = D // 2
                while w >= 2:
                    nxt = tpool.tile([128, RTILE * w], BF16, tag=f"T{h}{w}", name=f"T{h}{w}")
                    cv = cur[:, :nt * 2 * w].rearrange(
                        "p (t d) -> p t d", t=nt)
                    nc.vector.tensor_tensor(
                        out=nxt[:, :nt * w].rearrange(
                            "p (t d) -> p t d", t=nt),
                        in0=cv[:, :, 0:w],
                        in1=cv[:, :, w:2 * w],
                        op=ALU.add,
                    )
                    cur = nxt
                    w //= 2
                cv = cur[:, :nt * 2].rearrange("p (t d) -> p t d", t=nt)
                nc.vector.tensor_tensor(
                    out=s[:, hi * nt:hi * nt + nt].unsqueeze(2),
                    in0=cv[:, :, 0:1],
                    in1=cv[:, :, 1:2],
                    op=ALU.add,
                )

            # ---- softmax over metapaths --------------------------------
            sr = spool.tile([128, 2 * RTILE], BF16, tag="sr")
            nc.scalar.activation(sr[:, :2 * nt], s[:, :2 * nt], ACTF.Relu)
            e = spool.tile([128, 2 * RTILE], BF16, tag="e")
            nc.scalar.activation(e[:, :2 * nt], sr[:, :2 * nt], ACTF.Exp)

            sums = psum_s.tile([32, RTILE], F32, tag="sums")
            nc.tensor.matmul(out=sums[:, :nt], lhsT=blk4[:],
                             rhs=e[:, 0:nt], start=True, stop=False)
            nc.tensor.matmul(out=sums[:, :nt], lhsT=blk4[:],
                             rhs=e[:, nt:2 * nt], start=False, stop=True)
            inv = spool.tile([32, RTILE], F32, tag="inv")
            nc.vector.reciprocal(inv[:, :nt], sums[:, :nt])
            invb = psum_s.tile([128, RTILE], F32, tag="invb")
            nc.tensor.matmul(out=invb[:, :nt], lhsT=blk4t[:],
                             rhs=inv[:, :nt], start=True, stop=True)

            att = spool.tile([128, 2 * RTILE], BF16, tag="att")
            nc.vector.scalar_tensor_tensor(
                out=att[:, :2 * nt].rearrange("p (h t) -> p h t", h=2),
                in0=e[:, :2 * nt].rearrange("p (h t) -> p h t", h=2),
                scalar=1.0,
                in1=invb[:, :nt].unsqueeze(1).broadcast_to([128, 2, nt]),
                op0=ALU.mult, op1=ALU.mult,
            )

            # ---- pooling: scatter att into stationaries, PE matmuls ----
            S = {}
            for hi, h in enumerate(("a", "b")):
                S[h] = scat.tile([128, RTILE * 32], BF16, tag=f"S{h}", name=f"S{h}")
                nc.gpsimd.local_scatter(
                    S[h][:, :nt * 32], att[:, hi * nt:hi * nt + nt],
                    sidx[:, :nt], channels=128,
                    num_elems=nt * 32, num_idxs=nt)

            pool_ps = psum.tile([128, RTILE * 32], F32, tag="pool")
            for tt in range(nt):
                po = 32 * (tt & 3)
                co = D * (tt >> 2)
                nc.tensor.matmul(
                    out=pool_ps[po:po + 32, co:co + D],
                    lhsT=S["a"][:, 32 * tt:32 * tt + 32],
                    rhs=X["a"][:, D * tt:D * tt + D],
                    start=True, stop=False, tile_position=(0, po))
                nc.tensor.matmul(
                    out=pool_ps[po:po + 32, co:co + D],
                    lhsT=S["b"][:, 32 * tt:32 * tt + 32],
                    rhs=X["b"][:, D * tt:D * tt + D],
                    start=False, stop=True, tile_position=(0, po))

            # ---- elu(x) = relu(x) + exp(-relu(-x)) - 1 -----------------
            rl = epool.tile([128, RTILE * 32], BF16, tag="rl")
            nc.scalar.activation(rl[:, :nn], pool_ps[:, :nn], ACTF.Relu)
            t2 = epool.tile([128, RTILE * 32], BF16, tag="t2")
            nc.scalar.activation(t2[:, :nn], pool_ps[:, :nn], ACTF.Relu,
                                 scale=-1.0)
            e2 = epool.tile([128, RTILE * 32], BF16, tag="e2")
            nc.scalar.activation(e2[:, :nn], t2[:, :nn], ACTF.Exp,
                                 scale=-1.0)
            cmb = epool.tile([128, RTILE * 32], BF16, tag="cmb")
            nc.vector.tensor_tensor(
                out=cmb[:, :nn], in0=e2[:, :nn], in1=rl[:, :nn], op=ALU.add)
            out_sb = opool.tile([128, RTILE * 32], BF16, tag="osb")
            nc.vector.tensor_scalar(
                out_sb[:, :nn], cmb[:, :nn], -1.0, None, ALU.add)

            nc.sync.dma_start(out_d[:, t0 * 32:t0 * 32 + nn], out_sb[:, :nn])
            t0 += nt


def host_inputs(x_np, w_np):
    """Build per-core input maps from full fp32 inputs."""
    q = np.arange(128) >> 2          # node-in-tile per partition
    mi = np.arange(128) & 3          # metapath-within-half per partition

    wba = np.ascontiguousarray(w_np.T[mi, :]).astype(ml_dtypes.bfloat16)
    wbb = np.ascontiguousarray(w_np.T[4 + mi, :]).astype(ml_dtypes.bfloat16)
    blk4 = (np.arange(32)[None, :] == q[:, None]).astype(ml_dtypes.bfloat16)
    blk4t = np.ascontiguousarray(blk4.T).astype(np.float32)
    sidx = (32 * np.arange(RTILE)[None, :] + q[:, None]).astype(np.int16)

    in_maps = []
    for c in range(NCORES):
        xs = x_np[:, c * NC_RAW:(c + 1) * NC_RAW, :]
        xp = np.zeros((NMETA, NC_PAD, D), dtype=ml_dtypes.bfloat16)
        xp[:, :NC_RAW, :] = xs.astype(ml_dtypes.bfloat16)
        arr = xp.reshape(NMETA, NTILES, 32, D)
        # partition p = q*4 + mi  ->  [q, mi, t, d]
        xa = np.ascontiguousarray(
            arr[0:4].transpose(2, 0, 1, 3).reshape(128, NTILES, D))
        xb = np.ascontiguousarray(
            arr[4:8].transpose(2, 0, 1, 3).reshape(128, NTILES, D))
        in_maps.append({"xa": xa, "xb": xb, "wba": wba, "wbb": wbb,
                        "blk4": blk4, "blk4t": blk4t, "sidx": sidx})
    return in_maps


def unshard(res):
    """Per-core [128, NC_PAD] bf16 psum-slot layout -> full [N, D] f32."""
    full = np.empty((NCORES, NC_RAW, D), dtype=np.float32)
    sizes = _region_sizes()
    for c in range(NCORES):
        o = np.asarray(res.results[c]["out"]).astype(np.float32)
        parts = []
        col = 0
        for nt in sizes:
            nn = nt * 32
            b = o[:, col:col + nn].reshape(4, 32, nt // 4, D)
            # node-in-region = 32*(cblk*4 + pblk) + q
            parts.append(b.transpose(2, 0, 1, 3).reshape(nn, D))
            col += nn
        full[c] = np.concatenate(parts, axis=0)[:NC_RAW]
    return full.reshape(N, D)


_CACHE = {}


def build():
    if "nc" in _CACHE:
        return _CACHE["nc"]
    nc = bacc.Bacc("TRN2", target_bir_lowering=False, debug=False,
                   num_devices=NCORES)
    xa = nc.dram_tensor("xa", [128, NTILES, D], BF16, kind="ExternalInput").ap()
    xb = nc.dram_tensor("xb", [128, NTILES, D], BF16, kind="ExternalInput").ap()
    wba = nc.dram_tensor("wba", [128, D], BF16, kind="ExternalInput").ap()
    wbb = nc.dram_tensor("wbb", [128, D], BF16, kind="ExternalInput").ap()
    blk4 = nc.dram_tensor("blk4", [128, 32], BF16, kind="ExternalInput").ap()
    blk4t = nc.dram_tensor("blk4t", [32, 128], F32, kind="ExternalInput").ap()
    sidx = nc.dram_tensor("sidx", [128, RTILE], I16, kind="ExternalInput").ap()
    out = nc.dram_tensor("out", [128, NC_PAD], BF16, kind="ExternalOutput").ap()
    with tile.TileContext(nc) as tc:
        kernel_body(tc, out, xa, xb, wba, wbb, blk4, blk4t, sidx)
    nc.compile()
    _CACHE["nc"] = nc
    return nc


def run(input, W, trace=False, **trace_kwargs):
    x_np = np.asarray(input, dtype=np.float32)
    w_np = np.asarray(W, dtype=np.float32)
    nc = build()
    in_maps = host_inputs(x_np, w_np)
    res = bass_utils.run_bass_kernel_spmd(
        nc, in_maps, core_ids=list(range(NCORES)), trace=trace, **trace_kwargs)
    return unshard(res), res


def kernel(input, W):
    out, _ = run(input, W, trace=False)
    return out


# revision 7
# speedup vs baseline: 1.2863x; 1.2405x over previous
"""MetapathAttentionLayer Trainium2 kernel (v2: packed node-metapath layout).

Math (per node n):
    scores[n, m] = sum_d x[m, n, d] * W[d, m]
    att = softmax(relu(scores), axis=m)      (8 metapaths)
    out[n, :] = elu(sum_m att[n, m] * x[m, n, :])

Strategy: shard nodes across 8 cores (data parallel).  Per core, nodes are
packed so SBUF partition p = (node%32)*4 + metapath' holds one (node,
metapath) row of x, split into two halves (metapaths 0-3 / 4-7).  Per
region of 1024 nodes (32 tiles of 32 nodes):
  - scores: DVE tensor_tensor multiply against a replicated-W pattern
    (per-partition W column), then a batched binary-tree reduction over d
    (all tree levels are single DVE ops covering every tile).
  - softmax over m: ACT relu+exp; sum over the 8 metapaths of each node
    via PE matmul with a constant block-indicator stationary; DVE
    reciprocal; broadcast back with a second const matmul; weights applied
    on GPSIMD.
  - pooling: GPSIMD local_scatter packs attention weights into 32-wide
    stationaries (4 diagonals each); PE matmuls contract the (node,
    metapath) partition dim, accumulating both halves into PSUM.
  - elu(x) = relu(x) + exp(-relu(-x)) - 1: ACT x3 + GPSIMD combine,
    bf16 output DMA.
"""

from contextlib import ExitStack

import numpy as np
import ml_dtypes

import concourse.bass as bass
import concourse.tile as tile
from concourse import bacc, mybir, library_config
import concourse.bass_utils as bass_utils

F32 = mybir.dt.float32
BF16 = mybir.dt.bfloat16
I16 = mybir.dt.int16
ALU = mybir.AluOpType
ACTF = mybir.ActivationFunctionType

NMETA = 8
N = 100000
D = 128
NCORES = 8
NC_RAW = N // NCORES          # 12500 nodes per core
NC_PAD = 12800                # 400 tiles of 32 nodes
NTILES = NC_PAD // 32         # 400
RTILE = 32                    # tiles per region (1024 nodes)


def _region_sizes():
    """Tiles per region: 12 full regions of 32 tiles + 1 region of 16."""
    sizes = []
    t = NTILES
    while t > 0:
        s = min(RTILE, t)
        sizes.append(s)
        t -= s
    return sizes


def kernel_body(tc, out_d, xa_d, xb_d, wba_d, wbb_d, blk4_d, blk4t_d, sidx_d):
    nc = tc.nc
    sizes = _region_sizes()
    starts = [sum(sizes[:i]) for i in range(len(sizes))]
    R = len(sizes)
    with ExitStack() as ctx:
        const = ctx.enter_context(tc.tile_pool(name="const", bufs=1))
        xpool = ctx.enter_context(tc.tile_pool(name="x", bufs=4))
        ppool = ctx.enter_context(tc.tile_pool(name="prod", bufs=2))
        tpool = ctx.enter_context(tc.tile_pool(name="tree", bufs=2))
        spool = ctx.enter_context(tc.tile_pool(name="smalls", bufs=3))
        scat = ctx.enter_context(tc.tile_pool(name="scat", bufs=3))
        epool = ctx.enter_context(tc.tile_pool(name="elu", bufs=2))
        opool = ctx.enter_context(tc.tile_pool(name="osb", bufs=3))
        psum = ctx.enter_context(tc.tile_pool(name="ps", bufs=3, space="PSUM"))
        psum_s = ctx.enter_context(tc.tile_pool(name="pss", bufs=2, space="PSUM"))

        wba = const.tile([128, D], BF16)
        nc.sync.dma_start(wba[:], wba_d[:])
        wbb = const.tile([128, D], BF16)
        nc.sync.dma_start(wbb[:], wbb_d[:])
        blk4 = const.tile([128, 32], BF16)
        nc.sync.dma_start(blk4[:], blk4_d[:])
        blk4t = const.tile([32, 128], F32)
        nc.sync.dma_start(blk4t[:], blk4t_d[:])
        sidx = const.tile([128, RTILE], I16)
        nc.sync.dma_start(sidx[:], sidx_d[:])
        nc.gpsimd.load_library(library_config.local_scatter)

        st = {}   # region -> dict of live tiles

        def stage_dma(r):
            nt = sizes[r]
            fw = nt * D
            d = {"nt": nt}
            for h, x_d in (("a", xa_d), ("b", xb_d)):
                xt = xpool.tile([128, RTILE * D], BF16, tag=f"X{h}",
                                name=f"X{h}")
                nc.sync.dma_start(
                    xt[:, :fw],
                    x_d[:, starts[r]:starts[r] + nt, :].rearrange(
                        "p t d -> p (t d)"))
                d[f"X{h}"] = xt
            st[r] = d

        def stage_scores(r):
            """mult + tree + relu/exp + Σe matmuls (no recip/att yet)."""
            d = st[r]
            nt = d["nt"]
            fw = nt * D
            s = spool.tile([128, 2 * RTILE], F32, tag="s")
            for hi, (h, wb) in enumerate((("a", wba), ("b", wbb))):
                P = ppool.tile([128, RTILE * D], BF16, tag=f"P{h}",
                               name=f"P{h}")
                nc.vector.tensor_tensor(
                    out=P[:, :fw].rearrange("p (t d) -> p t d", t=nt),
                    in0=d[f"X{h}"][:, :fw].rearrange("p (t d) -> p t d", t=nt),
                    in1=wb[:].unsqueeze(1).broadcast_to([128, nt, D]),
                    op=ALU.mult,
                )
                cur = P
                w = D // 2
                while w >= 2:
                    nxt = tpool.tile([128, RTILE * w], BF16, tag=f"T{h}{w}",
                                     name=f"T{h}{w}")
                    cv = cur[:, :nt * 2 * w].rearrange(
                        "p (t d) -> p t d", t=nt)
                    nc.vector.tensor_tensor(
                        out=nxt[:, :nt * w].rearrange(
                            "p (t d) -> p t d", t=nt),
                        in0=cv[:, :, 0:w],
                        in1=cv[:, :, w:2 * w],
                        op=ALU.add,
                    )
                    cur = nxt
                    w //= 2
                cv = cur[:, :nt * 2].rearrange("p (t d) -> p t d", t=nt)
                nc.vector.tensor_tensor(
                    out=s[:, hi * nt:hi * nt + nt].unsqueeze(2),
                    in0=cv[:, :, 0:1],
                    in1=cv[:, :, 1:2],
                    op=ALU.add,
                )
            sr = spool.tile([128, 2 * RTILE], BF16, tag="sr")
            nc.scalar.activation(sr[:, :2 * nt], s[:, :2 * nt], ACTF.Relu)
            e = spool.tile([128, 2 * RTILE], BF16, tag="e")
            nc.scalar.activation(e[:, :2 * nt], sr[:, :2 * nt], ACTF.Exp)
            # sums into [0:32, 0:nt] of the shared small psum tile
            sm = psum_s.tile([128, 2 * RTILE], F32, tag="sm")
            nc.tensor.matmul(out=sm[0:32, 0:nt], lhsT=blk4[:],
                             rhs=e[:, 0:nt], start=True, stop=False)
            nc.tensor.matmul(out=sm[0:32, 0:nt], lhsT=blk4[:],
                             rhs=e[:, nt:2 * nt], start=False, stop=True)
            d["e"] = e
            d["sm"] = sm

        def stage_att(r):
            """recip + inv broadcast + att + scatter."""
            d = st[r]
            nt = d["nt"]
            e, sm = d["e"], d["sm"]
            inv = spool.tile([32, RTILE], F32, tag="inv")
            nc.vector.reciprocal(inv[:, :nt], sm[0:32, 0:nt])
            nc.tensor.matmul(out=sm[:, RTILE:RTILE + nt], lhsT=blk4t[:],
                             rhs=inv[:, :nt], start=True, stop=True)
            att = spool.tile([128, 2 * RTILE], BF16, tag="att")
            nc.vector.scalar_tensor_tensor(
                out=att[:, :2 * nt].rearrange("p (h t) -> p h t", h=2),
                in0=e[:, :2 * nt].rearrange("p (h t) -> p h t", h=2),
                scalar=1.0,
                in1=sm[:, RTILE:RTILE + nt].unsqueeze(1).broadcast_to(
                    [128, 2, nt]),
                op0=ALU.mult, op1=ALU.mult,
            )
            for hi, h in enumerate(("a", "b")):
                S = scat.tile([128, RTILE * 32], BF16, tag=f"S{h}",
                              name=f"S{h}")
                nc.gpsimd.local_scatter(
                    S[:, :nt * 32], att[:, hi * nt:hi * nt + nt],
                    sidx[:, :nt], channels=128,
                    num_elems=nt * 32, num_idxs=nt)
                d[f"S{h}"] = S

        def stage_pool(r):
            d = st[r]
            nt = d["nt"]
            pool_ps = psum.tile([128, RTILE * 32], F32, tag="pool")
            for tt in range(nt):
                po = 32 * (tt & 3)
                co = D * (tt >> 2)
                nc.tensor.matmul(
                    out=pool_ps[po:po + 32, co:co + D],
                    lhsT=d["Sa"][:, 32 * tt:32 * tt + 32],
                    rhs=d["Xa"][:, D * tt:D * tt + D],
                    start=True, stop=False, tile_position=(0, po))
                nc.tensor.matmul(
                    out=pool_ps[po:po + 32, co:co + D],
                    lhsT=d["Sb"][:, 32 * tt:32 * tt + 32],
                    rhs=d["Xb"][:, D * tt:D * tt + D],
                    start=False, stop=True, tile_position=(0, po))
            d["pool"] = pool_ps

        def stage_elu(r):
            d = st[r]
            nt = d["nt"]
            nn = nt * 32
            pool_ps = d["pool"]
            rl = epool.tile([128, RTILE * 32], BF16, tag="rl")
            nc.scalar.activation(rl[:, :nn], pool_ps[:, :nn], ACTF.Relu)
            t2 = epool.tile([128, RTILE * 32], BF16, tag="t2")
            nc.scalar.activation(t2[:, :nn], pool_ps[:, :nn], ACTF.Relu,
                                 scale=-1.0)
            e2 = epool.tile([128, RTILE * 32], BF16, tag="e2")
            nc.scalar.activation(e2[:, :nn], t2[:, :nn], ACTF.Exp,
                                 scale=-1.0)
            cmb = epool.tile([128, RTILE * 32], BF16, tag="cmb")
            nc.vector.tensor_tensor(
                out=cmb[:, :nn], in0=e2[:, :nn], in1=rl[:, :nn], op=ALU.add)
            out_sb = opool.tile([128, RTILE * 32], BF16, tag="osb")
            nc.vector.tensor_scalar(
                out_sb[:, :nn], cmb[:, :nn], -1.0, None, ALU.add)
            nc.sync.dma_start(
                out_d[:, starts[r] * 32:starts[r] * 32 + nn], out_sb[:, :nn])
            del st[r]

        # software pipeline: DMA r | scores r-1 | elu r-3 | att r-1 | pool r-2
        for k in range(R + 3):
            if k < R:
                stage_dma(k)
            if 1 <= k <= R:
                stage_scores(k - 1)
            if 3 <= k:
                stage_elu(k - 3)
            if 1 <= k <= R:
                stage_att(k - 1)
            if 2 <= k <= R + 1:
                stage_pool(k - 2)


def host_inputs(x_np, w_np):
    """Build per-core input maps from full fp32 inputs."""
    q = np.arange(128) >> 2          # node-in-tile per partition
    mi = np.arange(128) & 3          # metapath-within-half per partition

    wba = np.ascontiguousarray(w_np.T[mi, :]).astype(ml_dtypes.bfloat16)
    wbb = np.ascontiguousarray(w_np.T[4 + mi, :]).astype(ml_dtypes.bfloat16)
    blk4 = (np.arange(32)[None, :] == q[:, None]).astype(ml_dtypes.bfloat16)
    blk4t = np.ascontiguousarray(blk4.T).astype(np.float32)
    sidx = (32 * np.arange(RTILE)[None, :] + q[:, None]).astype(np.int16)

    in_maps = []
    for c in range(NCORES):
        xs = x_np[:, c * NC_RAW:(c + 1) * NC_RAW, :]
        xp = np.zeros((NMETA, NC_PAD, D), dtype=ml_dtypes.bfloat16)
        xp[:, :NC_RAW, :] = xs.astype(ml_dtypes.bfloat16)
        arr = xp.reshape(NMETA, NTILES, 32, D)
        # partition p = q*4 + mi  ->  [q, mi, t, d]
        xa = np.ascontiguousarray(
            arr[0:4].transpose(2, 0, 1, 3).reshape(128, NTILES, D))
        xb = np.ascontiguousarray(
            arr[4:8].transpose(2, 0, 1, 3).reshape(128, NTILES, D))
        in_maps.append({"xa": xa, "xb": xb, "wba": wba, "wbb": wbb,
                        "blk4": blk4, "blk4t": blk4t, "sidx": sidx})
    return in_maps


def unshard(res):
    """Per-core [128, NC_PAD] bf16 psum-slot layout -> full [N, D] f32."""
    full = np.empty((NCORES, NC_RAW, D), dtype=np.float32)
    sizes = _region_sizes()
    for c in range(NCORES):
        o = np.asarray(res.results[c]["out"]).astype(np.float32)
        parts = []
        col = 0
        for nt in sizes:
            nn = nt * 32
            b = o[:, col:col + nn].reshape(4, 32, nt // 4, D)
            # node-in-region = 32*(cblk*4 + pblk) + q
            parts.append(b.transpose(2, 0, 1, 3).reshape(nn, D))
            col += nn
        full[c] = np.concatenate(parts, axis=0)[:NC_RAW]
    return full.reshape(N, D)


_CACHE = {}


def build():
    if "nc" in _CACHE:
        return _CACHE["nc"]
    nc = bacc.Bacc("TRN2", target_bir_lowering=False, debug=False,
                   num_devices=NCORES)
    xa = nc.dram_tensor("xa", [128, NTILES, D], BF16, kind="ExternalInput").ap()
    xb = nc.dram_tensor("xb", [128, NTILES, D], BF16, kind="ExternalInput").ap()
    wba = nc.dram_tensor("wba", [128, D], BF16, kind="ExternalInput").ap()
    wbb = nc.dram_tensor("wbb", [128, D], BF16, kind="ExternalInput").ap()
    blk4 = nc.dram_tensor("blk4", [128, 32], BF16, kind="ExternalInput").ap()
    blk4t = nc.dram_tensor("blk4t", [32, 128], F32, kind="ExternalInput").ap()
    sidx = nc.dram_tensor("sidx", [128, RTILE], I16, kind="ExternalInput").ap()
    out = nc.dram_tensor("out", [128, NC_PAD], BF16, kind="ExternalOutput").ap()
    with tile.TileContext(nc) as tc:
        kernel_body(tc, out, xa, xb, wba, wbb, blk4, blk4t, sidx)
    nc.compile()
    _CACHE["nc"] = nc
    return nc


def run(input, W, trace=False, **trace_kwargs):
    x_np = np.asarray(input, dtype=np.float32)
    w_np = np.asarray(W, dtype=np.float32)
    nc = build()
    in_maps = host_inputs(x_np, w_np)
    res = bass_utils.run_bass_kernel_spmd(
        nc, in_maps, core_ids=list(range(NCORES)), trace=trace, **trace_kwargs)
    return unshard(res), res


def kernel(input, W):
    out, _ = run(input, W, trace=False)
    return out
